# revision 1
# baseline (speedup 1.0000x reference)
"""DeepRedModel co-attention forward on 8 trn2 NeuronCores (Bass/Tile).

Data-parallel over the batch dim: each core handles B/8 = 128 batch rows,
embedding table replicated (in HBM, gathered via indirect DMA).

Per-core dataflow (BP = 128 batches):
  - host packs gather indices [128, 2*BP] int32: col 2b -> positions 0:128 of
    batch b, col 2b+1 -> positions 128:200 (partitions 72:128 padded OOB).
  - bulk indirect-DMA gathers bf16 table rows into resident SBUF tiles
    UG/IG [128p, 2*BP, 128d].
  - per batch b: PE transposes (matmul vs identity) -> T=[d, s|t] bf16;
    align = T_u^T @ T_i on PE ([s,t], fp32 PSUM); tanh on ScalarE with
    accum_out giving user coef sums; item coef sums via matmul with a
    sliding ones-column stationary, accumulating rows [b, t] in PSUM.
  - batched softmax over [BP, 200] (both sides), PE transpose of weights
    back to [s, b] columns.
  - pass 2: per batch, PE matvec rep = U^T w accumulated into [d, b] PSUM
    columns; final PE transpose -> [b, d]; DMA out [2, BP, 128] fp32.
"""

import numpy as np

_B, _S, _V, _D = 1024, 200, 100000, 128
_NC = 8
_BP = _B // _NC
_PAD = 1 << 28  # OOB pad index (> bounds_check) -> gather skipped
_GCOLS = 32     # index columns per indirect gather (16 batches)


def build_bass(BP=_BP, gcols=_GCOLS):
    import concourse.bass as bass
    import concourse.mybir as mybir
    import concourse.tile as tile
    from concourse import bacc
    from concourse.masks import make_identity

    f32 = mybir.dt.float32
    bf16 = mybir.dt.bfloat16
    i32 = mybir.dt.int32
    S, V, D = _S, _V, _D
    S1 = 128          # first position chunk
    S2 = S - S1       # 72
    NCOL = 2 * BP     # index/embedding columns per side

    nc = bacc.Bacc("TRN2", target_bir_lowering=False, debug=False)

    table = nc.dram_tensor("table", [V + 1, D], bf16, kind="ExternalInput")
    uidx = nc.dram_tensor("uidx", [128, NCOL], i32, kind="ExternalInput")
    iidx = nc.dram_tensor("iidx", [128, NCOL], i32, kind="ExternalInput")
    umask = nc.dram_tensor("umask", [BP, S], f32, kind="ExternalInput")
    imask = nc.dram_tensor("imask", [BP, S], f32, kind="ExternalInput")
    out_t = nc.dram_tensor("out", [2, BP, D], f32, kind="ExternalOutput")

    with tile.TileContext(nc) as tc:
        with (
            tc.tile_pool(name="const", bufs=1) as cpool,
            tc.tile_pool(name="emb", bufs=1) as epool,
            tc.tile_pool(name="coef", bufs=1) as coefpool,
            tc.tile_pool(name="soft", bufs=1) as spool,
            tc.tile_pool(name="tps", bufs=2, space="PSUM") as tpsum,
            tc.tile_pool(name="aps", bufs=2, space="PSUM") as apsum,
            tc.tile_pool(name="icps", bufs=1, space="PSUM") as icpsum,
            tc.tile_pool(name="wps", bufs=1, space="PSUM") as wpsum,
            tc.tile_pool(name="t16p", bufs=3) as t16pool,
            tc.tile_pool(name="thp", bufs=3) as thpool,
        ):
            # ---- constants ----
            id16 = cpool.tile([128, 128], bf16)
            id32 = cpool.tile([128, 128], f32)
            make_identity(nc, id16[:])
            make_identity(nc, id32[:])
            # ones-column tile: C[s, c] = 1.0 iff c == 127 (sliding window
            # C[:, 127-b:255-b] has its ones column at position b)
            cones = cpool.tile([128, 256], bf16)
            nc.gpsimd.memset(cones[:], 0.0)
            nc.gpsimd.memset(cones[:, 127:128], 1.0)

            # ---- load indices + masks ----
            uidx_t = cpool.tile([128, NCOL], i32, tag="uidx")
            iidx_t = cpool.tile([128, NCOL], i32, tag="iidx")
            umask_t = cpool.tile([BP, S], f32, tag="umask")
            imask_t = cpool.tile([BP, S], f32, tag="imask")
            nc.sync.dma_start(out=uidx_t[:], in_=uidx[:])
            nc.sync.dma_start(out=iidx_t[:], in_=iidx[:])
            nc.sync.dma_start(out=umask_t[:], in_=umask[:])
            nc.sync.dma_start(out=imask_t[:], in_=imask[:])

            # ---- bulk gathers: table rows -> [128, NCOL, D] bf16 ----
            ug = epool.tile([128, NCOL, D], bf16, tag="ug")
            ig = epool.tile([128, NCOL, D], bf16, tag="ig")
            for side_idx, side_g in ((uidx_t, ug), (iidx_t, ig)):
                for g0 in range(NCOL):
                    nc.gpsimd.indirect_dma_start(
                        out=side_g[:, g0, :],
                        out_offset=None,
                        in_=table[:],
                        in_offset=bass.IndirectOffsetOnAxis(
                            ap=side_idx[:, g0:g0 + 1], axis=0
                        ),
                        bounds_check=V,
                        oob_is_err=False,
                    )

            # ---- pass 1: per-batch align/tanh/coefs ----
            ucc1 = coefpool.tile([128, BP], f32, tag="ucc1")  # user coef s=0:128
            ucc2 = coefpool.tile([128, BP], f32, tag="ucc2")  # user coef s=128:200
            icp = icpsum.tile([128, 512], f32, tag="ic")      # item coef rows [b, t]

            for b in range(BP):
                tp = tpsum.tile([128, 512], f32, tag="tp")
                # transposes: T_u -> cols 0:200, T_i -> cols 200:400
                nc.tensor.matmul(out=tp[:, 0:S1], lhsT=ug[:, 2 * b, :],
                                 rhs=id16[:], start=True, stop=True)
                nc.tensor.matmul(out=tp[:, S1:S], lhsT=ug[0:S2, 2 * b + 1, :],
                                 rhs=id16[0:S2, 0:S2], start=True, stop=True)
                nc.tensor.matmul(out=tp[:, S:S + S1], lhsT=ig[:, 2 * b, :],
                                 rhs=id16[:], start=True, stop=True)
                nc.tensor.matmul(out=tp[:, S + S1:2 * S], lhsT=ig[0:S2, 2 * b + 1, :],
                                 rhs=id16[0:S2, 0:S2], start=True, stop=True)
                t16 = t16pool.tile([128, 2 * S], bf16, tag="t16")
                nc.vector.tensor_copy(out=t16[:], in_=tp[:, 0:2 * S])

                # align[s, t] = sum_d T_u[d, s] * T_i[d, t]
                ap_ = apsum.tile([128, 512], f32, tag="ap")
                nc.tensor.matmul(out=ap_[:, 0:S], lhsT=t16[:, 0:S1],
                                 rhs=t16[:, S:2 * S], start=True, stop=True)
                nc.tensor.matmul(out=ap_[0:S2, S:2 * S], lhsT=t16[:, S1:S],
                                 rhs=t16[:, S:2 * S], start=True, stop=True)

                # tanh + user-coef row sums (accum_out)
                th1 = thpool.tile([128, S], bf16, tag="th1")
                th2 = thpool.tile([S2, S], bf16, tag="th2")
                nc.scalar.activation(out=th1[:], in_=ap_[:, 0:S],
                                     func=mybir.ActivationFunctionType.Tanh,
                                     accum_out=ucc1[:, b:b + 1])
                nc.scalar.activation(out=th2[:], in_=ap_[0:S2, S:2 * S],
                                     func=mybir.ActivationFunctionType.Tanh,
                                     accum_out=ucc2[0:S2, b:b + 1])

                # item coef: row b of icp accumulates ones^T @ tanh
                nc.tensor.matmul(out=icp[:, 0:S], lhsT=cones[:, 127 - b:255 - b],
                                 rhs=th1[:], start=(b == 0), stop=False,
                                 skip_group_check=True)
                nc.tensor.matmul(out=icp[:, 0:S], lhsT=cones[0:S2, 127 - b:255 - b],
                                 rhs=th2[:], start=False, stop=(b == BP - 1),
                                 skip_group_check=True)

            # ---- softmax (batched over BP rows) ----
            # user coefs: transpose ucc [s, b] -> [b, s]
            ucp = wpsum.tile([128, 512], f32, tag="wps")
            nc.tensor.matmul(out=ucp[0:BP, 0:S1], lhsT=ucc1[:], rhs=id32[:],
                             start=True, stop=True)
            nc.tensor.matmul(out=ucp[0:BP, S1:S], lhsT=ucc2[0:S2, :],
                             rhs=id32[0:S2, 0:S2], start=True, stop=True)

            wu_sb = spool.tile([BP, S], f32, tag="wu")
            wi_sb = spool.tile([BP, S], f32, tag="wi")
            for coef_psum, mask_t, w_sb in (
                (ucp[0:BP, 0:S], umask_t, wu_sb),
                (icp[0:BP, 0:S], imask_t, wi_sb),
            ):
                y0 = spool.tile([BP, S], f32, tag="y0")
                y1 = spool.tile([BP, S], f32, tag="y1")
                mx = spool.tile([BP, 1], f32, tag="mx")
                se = spool.tile([BP, 1], f32, tag="se")
                rse = spool.tile([BP, 1], f32, tag="rse")
                # y = coef_sum/200 + mask  (mask pre-scaled by 200 on host)
                nc.vector.tensor_scalar(out=y0[:], in0=coef_psum, scalar1=1.0 / S,
                                        scalar2=None, op0=mybir.AluOpType.mult)
                nc.vector.tensor_tensor(out=y1[:], in0=y0[:], in1=mask_t[:],
                                        op=mybir.AluOpType.add)
                nc.vector.reduce_max(out=mx[:], in_=y1[:],
                                     axis=mybir.AxisListType.X)
                nc.vector.tensor_scalar(out=y0[:], in0=y1[:], scalar1=mx[:],
                                        scalar2=None,
                                        op0=mybir.AluOpType.subtract)
                nc.scalar.activation(out=y1[:], in_=y0[:],
                                     func=mybir.ActivationFunctionType.Exp,
                                     accum_out=se[:])
                nc.vector.reciprocal(out=rse[:], in_=se[:])
                nc.vector.tensor_scalar(out=w_sb[:], in0=y1[:], scalar1=rse[:],
                                        scalar2=None, op0=mybir.AluOpType.mult)

            # transpose weights back to columns: [s, b] bf16
            wtp = wpsum.tile([128, 512], f32, tag="wps")
            nc.tensor.matmul(out=wtp[:, 0:BP], lhsT=wu_sb[:, 0:S1],
                             rhs=id32[0:BP, 0:BP], start=True, stop=True)
            nc.tensor.matmul(out=wtp[0:S2, 128:128 + BP], lhsT=wu_sb[:, S1:S],
                             rhs=id32[0:BP, 0:BP], start=True, stop=True)
            nc.tensor.matmul(out=wtp[:, 256:256 + BP], lhsT=wi_sb[:, 0:S1],
                             rhs=id32[0:BP, 0:BP], start=True, stop=True)
            nc.tensor.matmul(out=wtp[0:S2, 384:384 + BP], lhsT=wi_sb[:, S1:S],
                             rhs=id32[0:BP, 0:BP], start=True, stop=True)
            wu1 = spool.tile([128, BP], bf16, tag="wu1")
            wu2 = spool.tile([S2, BP], bf16, tag="wu2")
            wi1 = spool.tile([128, BP], bf16, tag="wi1")
            wi2 = spool.tile([S2, BP], bf16, tag="wi2")
            nc.vector.tensor_copy(out=wu1[:], in_=wtp[:, 0:BP])
            nc.vector.tensor_copy(out=wu2[:], in_=wtp[0:S2, 128:128 + BP])
            nc.vector.tensor_copy(out=wi1[:], in_=wtp[:, 256:256 + BP])
            nc.vector.tensor_copy(out=wi2[:], in_=wtp[0:S2, 384:384 + BP])

            # ---- pass 2: weighted sums rep[d] = sum_s U[s,d] w[s] ----
            repp = icpsum.tile([128, 512], f32, tag="rep")
            for b in range(BP):
                nc.tensor.matmul(out=repp[:, b:b + 1], lhsT=ug[:, 2 * b, :],
                                 rhs=wu1[:, b:b + 1], start=True, stop=False,
                                 skip_group_check=True)
                nc.tensor.matmul(out=repp[:, b:b + 1], lhsT=ug[0:S2, 2 * b + 1, :],
                                 rhs=wu2[:, b:b + 1], start=False, stop=True,
                                 skip_group_check=True)
                nc.tensor.matmul(out=repp[:, BP + b:BP + b + 1],
                                 lhsT=ig[:, 2 * b, :], rhs=wi1[:, b:b + 1],
                                 start=True, stop=False, skip_group_check=True)
                nc.tensor.matmul(out=repp[:, BP + b:BP + b + 1],
                                 lhsT=ig[0:S2, 2 * b + 1, :], rhs=wi2[:, b:b + 1],
                                 start=False, stop=True, skip_group_check=True)

            # ---- final transpose [d, b] -> [b, d] and store ----
            repsb = spool.tile([128, 2 * BP], f32, tag="repsb")
            nc.vector.tensor_copy(out=repsb[:], in_=repp[:, 0:2 * BP])
            outp = wpsum.tile([128, 512], f32, tag="wps")
            nc.tensor.matmul(out=outp[0:BP, 0:D], lhsT=repsb[:, 0:BP], rhs=id32[:],
                             start=True, stop=True)
            nc.tensor.matmul(out=outp[0:BP, D:2 * D], lhsT=repsb[:, BP:2 * BP],
                             rhs=id32[:], start=True, stop=True)
            outsb = spool.tile([BP, 2 * D], f32, tag="outsb")
            nc.vector.tensor_copy(out=outsb[:], in_=outp[0:BP, 0:2 * D])
            nc.sync.dma_start(out=out_t[0], in_=outsb[:, 0:D])
            nc.sync.dma_start(out=out_t[1], in_=outsb[:, D:2 * D])

    nc.finalize()
    return nc


_NQ = 6400                      # rows per quarter-side (32 batches x 200)
_NCW = [2304, 2304, 2304, 256]  # static per-window slot budgets
_CWS = [0, 2304, 4608, 6912, 7168]
_SLOTS = 7168


def build_bass_v2():
    """Two-stage gather variant: windowed int16 dma_gather -> sorted SBUF
    staging -> SBUF-source transposing dma_gather (un-permute), landing
    embeddings directly in [d, s] layout. Manual DMA-completion semaphores
    (dma_gather is not Tile-tracked) with dep pins."""
    import concourse.bass as bass
    import concourse.mybir as mybir
    import concourse.tile as tile
    from concourse import bacc
    from concourse.bass import _add_dep_helper as add_dep
    from concourse.masks import make_identity

    f32 = mybir.dt.float32
    bf16 = mybir.dt.bfloat16
    i16 = mybir.dt.int16
    BP, S, V, D = _BP, _S, _V, _D
    S1, S2 = 128, _S - 128
    NQ, SLOTS = _NQ, _SLOTS

    nc = bacc.Bacc("TRN2", target_bir_lowering=False, debug=False)

    table = nc.dram_tensor("table", [V + 1, D], bf16, kind="ExternalInput")
    gidx = nc.dram_tensor("gidx", [128, 8 * (SLOTS // 16)], i16, kind="ExternalInput")
    pidx = nc.dram_tensor("pidx", [128, 8 * (NQ // 16)], i16, kind="ExternalInput")
    umask = nc.dram_tensor("umask", [BP, S], f32, kind="ExternalInput")
    imask = nc.dram_tensor("imask", [BP, S], f32, kind="ExternalInput")
    out_t = nc.dram_tensor("out", [2, BP, D], f32, kind="ExternalOutput")

    gsem = [nc.alloc_semaphore(f"gsem{i}") for i in range(8)]
    dsem = [nc.alloc_semaphore(f"dsem{i}") for i in range(8)]

    with tile.TileContext(nc) as tc:
        with (
            tc.tile_pool(name="const", bufs=1) as cpool,
            tc.tile_pool(name="stgp", bufs=2) as stgpool,
            tc.tile_pool(name="tallp", bufs=1) as tallpool,
            tc.tile_pool(name="coef", bufs=1) as coefpool,
            tc.tile_pool(name="soft", bufs=1) as spool,
            tc.tile_pool(name="aps", bufs=2, space="PSUM") as apsum,
            tc.tile_pool(name="tps", bufs=2, space="PSUM") as tpsum,
            tc.tile_pool(name="icps", bufs=1, space="PSUM") as icpsum,
            tc.tile_pool(name="wps", bufs=1, space="PSUM") as wpsum,
            tc.tile_pool(name="thp", bufs=3) as thpool,
            tc.tile_pool(name="trashp", bufs=2) as trashpool,
        ):
            id32 = cpool.tile([128, 128], f32)
            make_identity(nc, id32[:])
            id16 = cpool.tile([128, 128], bf16)
            make_identity(nc, id16[:])
            cones = cpool.tile([128, 256], bf16)
            nc.gpsimd.memset(cones[:], 0.0)
            nc.gpsimd.memset(cones[:, 127:128], 1.0)

            gidx_t = cpool.tile([128, 8 * (SLOTS // 16)], i16, tag="gidx")
            pidx_t = cpool.tile([128, 8 * (NQ // 16)], i16, tag="pidx")
            umask_t = cpool.tile([BP, S], f32, tag="umask")
            imask_t = cpool.tile([BP, S], f32, tag="imask")
            in_loads = [
                nc.sync.dma_start(out=gidx_t[:], in_=gidx[:]),
                nc.sync.dma_start(out=pidx_t[:], in_=pidx[:]),
                nc.sync.dma_start(out=umask_t[:], in_=umask[:]),
                nc.sync.dma_start(out=imask_t[:], in_=imask[:]),
            ]
            # manual sems are NOT cleared by allocation; clear before use
            sem_clears = [nc.gpsimd.sem_clear(s) for s in gsem + dsem]

            tall = []
            for s_ in (0, 1):
                tall_s = tallpool.tile([128, 4, NQ], bf16, tag=f"tall{s_}")
                tall.append(tall_s)

            # ---- gather chain (serialized on POOL via sems) ----
            wd_of = {}
            prev_wd = None
            for q in range(4):
                for side in (0, 1):
                    qs = q * 2 + side
                    stg = stgpool.tile([128, SLOTS // 128, D], bf16, tag="stg")
                    gs = []
                    for c in range(4):
                        wbase = 32768 * c
                        wend = min(32768 * (c + 1), V + 1)
                        g = nc.gpsimd.dma_gather(
                            stg[:, _CWS[c] // 128:_CWS[c + 1] // 128, :],
                            table[wbase:wend, :],
                            gidx_t[:, qs * (SLOTS // 16) + _CWS[c] // 16:
                                   qs * (SLOTS // 16) + _CWS[c + 1] // 16],
                            _NCW[c],
                            _NCW[c],
                            D,
                        )
                        g.then_inc(gsem[qs], 16)
                        if prev_wd is not None:
                            add_dep(g.ins, prev_wd.ins, sync=False,
                                    reason="serialize DMA chain")
                        else:
                            # first quarter: sems cleared + input loads landed
                            for cl in sem_clears:
                                add_dep(g.ins, cl.ins, sync=False,
                                        reason="sem clear first")
                            for ld in in_loads:
                                add_dep(g.ins, ld.ins, sync=True,
                                        reason="no HWDGE during xbar gather")
                        gs.append(g)
                    wg = nc.gpsimd.wait_ge(gsem[qs], 16 * 4)
                    for g in gs:
                        add_dep(wg.ins, g.ins, sync=False, reason="wait after gathers")
                    up = nc.gpsimd.dma_gather(
                        tall[side][:, q:q + 1, :],
                        stg[:],
                        pidx_t[:, qs * (NQ // 16):(qs + 1) * (NQ // 16)],
                        NQ,
                        NQ,
                        D,
                        transpose=True,
                        sbuf_tokens_per_rank=128,
                        sbuf_free_dim_per_rank=D * 2,
                    )
                    add_dep(up.ins, wg.ins, sync=False, reason="unpermute after wait")
                    up.then_inc(dsem[qs], 16)
                    wd = nc.gpsimd.wait_ge(dsem[qs], 16)
                    add_dep(wd.ins, up.ins, sync=False, reason="wait after unpermute")
                    wd_of[(side, q)] = wd
                    prev_wd = wd

            # ---- pass 1 ----
            ucc1 = coefpool.tile([128, BP], f32, tag="ucc1")
            ucc2 = coefpool.tile([128, BP], f32, tag="ucc2")
            icp = icpsum.tile([128, 512], f32, tag="ic")

            for b in range(BP):
                q, r = b // 32, b % 32
                tu = tall[0][:, q, 200 * r:200 * r + 200]
                ti = tall[1][:, q, 200 * r:200 * r + 200]
                ap_ = apsum.tile([128, 512], f32, tag="ap")
                m1 = nc.tensor.matmul(out=ap_[:, 0:S], lhsT=tall[0][:, q, 200 * r:200 * r + S1],
                                      rhs=ti, start=True, stop=True)
                m2 = nc.tensor.matmul(out=ap_[0:S2, S:2 * S],
                                      lhsT=tall[0][:, q, 200 * r + S1:200 * r + S],
                                      rhs=ti, start=True, stop=True)
                for m in (m1, m2):
                    add_dep(m.ins, wd_of[(0, q)].ins, sync=True, reason="tall_u ready")
                    add_dep(m.ins, wd_of[(1, q)].ins, sync=True, reason="tall_i ready")

                th1 = thpool.tile([128, S], bf16, tag="th1")
                th2 = thpool.tile([S2, S], bf16, tag="th2")
                nc.scalar.activation(out=th1[:], in_=ap_[:, 0:S],
                                     func=mybir.ActivationFunctionType.Tanh,
                                     accum_out=ucc1[:, b:b + 1])
                nc.scalar.activation(out=th2[:], in_=ap_[0:S2, S:2 * S],
                                     func=mybir.ActivationFunctionType.Tanh,
                                     accum_out=ucc2[0:S2, b:b + 1])
                nc.tensor.matmul(out=icp[:, 0:S], lhsT=cones[:, 127 - b:255 - b],
                                 rhs=th1[:], start=(b == 0), stop=False,
                                 skip_group_check=True)
                nc.tensor.matmul(out=icp[:, 0:S], lhsT=cones[0:S2, 127 - b:255 - b],
                                 rhs=th2[:], start=False, stop=(b == BP - 1),
                                 skip_group_check=True)

            # ---- softmax ----
            ucp = wpsum.tile([128, 512], f32, tag="wps")
            nc.tensor.matmul(out=ucp[0:BP, 0:S1], lhsT=ucc1[:], rhs=id32[:],
                             start=True, stop=True)
            nc.tensor.matmul(out=ucp[0:BP, S1:S], lhsT=ucc2[0:S2, :],
                             rhs=id32[0:S2, 0:S2], start=True, stop=True)

            wu_sb = spool.tile([BP, S], f32, tag="wu")
            wi_sb = spool.tile([BP, S], f32, tag="wi")
            for coef_psum, mask_t, w_sb in (
                (ucp[0:BP, 0:S], umask_t, wu_sb),
                (icp[0:BP, 0:S], imask_t, wi_sb),
            ):
                y0 = spool.tile([BP, S], f32, tag="y0")
                y1 = spool.tile([BP, S], f32, tag="y1")
                mx = spool.tile([BP, 1], f32, tag="mx")
                se = spool.tile([BP, 1], f32, tag="se")
                rse = spool.tile([BP, 1], f32, tag="rse")
                nc.vector.tensor_scalar(out=y0[:], in0=coef_psum, scalar1=1.0 / S,
                                        scalar2=None, op0=mybir.AluOpType.mult)
                nc.vector.tensor_tensor(out=y1[:], in0=y0[:], in1=mask_t[:],
                                        op=mybir.AluOpType.add)
                nc.vector.reduce_max(out=mx[:], in_=y1[:], axis=mybir.AxisListType.X)
                nc.vector.tensor_scalar(out=y0[:], in0=y1[:], scalar1=mx[:],
                                        scalar2=None, op0=mybir.AluOpType.subtract)
                nc.scalar.activation(out=y1[:], in_=y0[:],
                                     func=mybir.ActivationFunctionType.Exp,
                                     accum_out=se[:])
                nc.vector.reciprocal(out=rse[:], in_=se[:])
                nc.vector.tensor_scalar(out=w_sb[:], in0=y1[:], scalar1=rse[:],
                                        scalar2=None, op0=mybir.AluOpType.mult)

            # transpose weights to columns [s, b] bf16 (v1 pattern)
            wtp = wpsum.tile([128, 512], f32, tag="wps")
            nc.tensor.matmul(out=wtp[:, 0:BP], lhsT=wu_sb[:, 0:S1],
                             rhs=id32[0:BP, 0:BP], start=True, stop=True)
            nc.tensor.matmul(out=wtp[0:S2, 128:128 + BP], lhsT=wu_sb[:, S1:S],
                             rhs=id32[0:BP, 0:BP], start=True, stop=True)
            nc.tensor.matmul(out=wtp[:, 256:256 + BP], lhsT=wi_sb[:, 0:S1],
                             rhs=id32[0:BP, 0:BP], start=True, stop=True)
            nc.tensor.matmul(out=wtp[0:S2, 384:384 + BP], lhsT=wi_sb[:, S1:S],
                             rhs=id32[0:BP, 0:BP], start=True, stop=True)
            wu1 = spool.tile([128, BP], bf16, tag="wu1")
            wu2 = spool.tile([S2, BP], bf16, tag="wu2")
            wi1 = spool.tile([128, BP], bf16, tag="wi1")
            wi2 = spool.tile([S2, BP], bf16, tag="wi2")
            nc.vector.tensor_copy(out=wu1[:], in_=wtp[:, 0:BP])
            nc.vector.tensor_copy(out=wu2[:], in_=wtp[0:S2, 128:128 + BP])
            nc.vector.tensor_copy(out=wi1[:], in_=wtp[:, 256:256 + BP])
            nc.vector.tensor_copy(out=wi2[:], in_=wtp[0:S2, 384:384 + BP])

            # ---- pass 2: transpose T back to [s, d], then PE matvecs ----
            repp = icpsum.tile([128, 512], f32, tag="rep")
            for b in range(BP):
                q, r = b // 32, b % 32
                tp = tpsum.tile([128, 512], f32, tag="tp")
                # layout: Uc0 [128p, 0:D], Ic0 [128p, D:2D], Uc1 [72p, 2D:3D],
                # Ic1 [72p, 3D:4D] -> two contiguous DVE copies
                t1 = nc.tensor.matmul(out=tp[:, 0:D], lhsT=tall[0][:, q, 200 * r:200 * r + S1],
                                      rhs=id16[:], start=True, stop=True)
                t3 = nc.tensor.matmul(out=tp[:, D:2 * D],
                                      lhsT=tall[1][:, q, 200 * r:200 * r + S1],
                                      rhs=id16[:], start=True, stop=True)
                t2 = nc.tensor.matmul(out=tp[0:S2, 2 * D:3 * D],
                                      lhsT=tall[0][:, q, 200 * r + S1:200 * r + S],
                                      rhs=id16[:], start=True, stop=True)
                t4 = nc.tensor.matmul(out=tp[0:S2, 3 * D:4 * D],
                                      lhsT=tall[1][:, q, 200 * r + S1:200 * r + S],
                                      rhs=id16[:], start=True, stop=True)
                for t_, sd in ((t1, 0), (t2, 0), (t3, 1), (t4, 1)):
                    add_dep(t_.ins, wd_of[(sd, q)].ins, sync=True, reason="tall ready")
                ug16 = trashpool.tile([128, 512], bf16, tag="ug16")
                nc.vector.tensor_copy(out=ug16[:, 0:2 * D], in_=tp[:, 0:2 * D])
                nc.vector.tensor_copy(out=ug16[0:S2, 2 * D:4 * D],
                                      in_=tp[0:S2, 2 * D:4 * D])
                nc.tensor.matmul(out=repp[:, b:b + 1], lhsT=ug16[:, 0:D],
                                 rhs=wu1[:, b:b + 1], start=True, stop=False,
                                 skip_group_check=True)
                nc.tensor.matmul(out=repp[:, b:b + 1], lhsT=ug16[0:S2, 2 * D:3 * D],
                                 rhs=wu2[:, b:b + 1], start=False, stop=True,
                                 skip_group_check=True)
                nc.tensor.matmul(out=repp[:, BP + b:BP + b + 1], lhsT=ug16[:, D:2 * D],
                                 rhs=wi1[:, b:b + 1], start=True, stop=False,
                                 skip_group_check=True)
                nc.tensor.matmul(out=repp[:, BP + b:BP + b + 1],
                                 lhsT=ug16[0:S2, 3 * D:4 * D],
                                 rhs=wi2[:, b:b + 1], start=False, stop=True,
                                 skip_group_check=True)

            # ---- final transpose + store ----
            repsb = spool.tile([128, 2 * BP], f32, tag="repsb")
            nc.vector.tensor_copy(out=repsb[:], in_=repp[:, 0:2 * BP])
            outp = wpsum.tile([128, 512], f32, tag="wps")
            nc.tensor.matmul(out=outp[0:BP, 0:D], lhsT=repsb[:, 0:BP], rhs=id32[:],
                             start=True, stop=True)
            nc.tensor.matmul(out=outp[0:BP, D:2 * D], lhsT=repsb[:, BP:2 * BP],
                             rhs=id32[:], start=True, stop=True)
            outsb = spool.tile([BP, 2 * D], f32, tag="outsb")
            nc.vector.tensor_copy(out=outsb[:], in_=outp[0:BP, 0:2 * D])
            nc.sync.dma_start(out=out_t[0], in_=outsb[:, 0:D])
            nc.sync.dma_start(out=out_t[1], in_=outsb[:, D:2 * D])

    nc.finalize()
    return nc


def build_bass_v3():
    """v1 compute + bulk gather: windowed dma_gather -> SBUF staging -> DRAM
    bounce -> plain position dma_gather into [s, d] batch-slot layout. Avoids
    the xbar-transpose mode entirely; every primitive HW-validated."""
    import concourse.bass as bass
    import concourse.mybir as mybir
    import concourse.tile as tile
    from concourse import bacc
    from concourse.bass import _add_dep_helper as add_dep
    from concourse.masks import make_identity

    f32 = mybir.dt.float32
    bf16 = mybir.dt.bfloat16
    i16 = mybir.dt.int16
    BP, S, V, D = _BP, _S, _V, _D
    S1, S2 = 128, _S - 128
    SLOTS = _SLOTS
    NP2 = 8192  # stage-2 positions per quarter-side (32 batches x 2 cols x 128)

    nc = bacc.Bacc("TRN2", target_bir_lowering=False, debug=False)
    table = nc.dram_tensor("table", [V + 1, D], bf16, kind="ExternalInput")
    gidx = nc.dram_tensor("gidx", [128, 8 * (SLOTS // 16)], i16, kind="ExternalInput")
    pidx = nc.dram_tensor("pidx", [128, 8 * (NP2 // 16)], i16, kind="ExternalInput")
    umask = nc.dram_tensor("umask", [BP, S], f32, kind="ExternalInput")
    imask = nc.dram_tensor("imask", [BP, S], f32, kind="ExternalInput")
    out_t = nc.dram_tensor("out", [2, BP, D], f32, kind="ExternalOutput")
    scrs = [nc.dram_tensor(f"scr{i}", [128, SLOTS // 128, D], bf16) for i in range(8)]

    gsem = [nc.alloc_semaphore(f"gsem{i}") for i in range(8)]
    dsem = [nc.alloc_semaphore(f"dsem{i}") for i in range(8)]

    with tile.TileContext(nc) as tc:
        with (
            tc.tile_pool(name="const", bufs=1) as cpool,
            tc.tile_pool(name="emb", bufs=1) as epool,
            tc.tile_pool(name="stgp", bufs=2) as stgpool,
            tc.tile_pool(name="coef", bufs=1) as coefpool,
            tc.tile_pool(name="soft", bufs=1) as spool,
            tc.tile_pool(name="tps", bufs=2, space="PSUM") as tpsum,
            tc.tile_pool(name="aps", bufs=2, space="PSUM") as apsum,
            tc.tile_pool(name="icps", bufs=1, space="PSUM") as icpsum,
            tc.tile_pool(name="wps", bufs=1, space="PSUM") as wpsum,
            tc.tile_pool(name="t16p", bufs=3) as t16pool,
            tc.tile_pool(name="thp", bufs=3) as thpool,
        ):
            id16 = cpool.tile([128, 128], bf16)
            id32 = cpool.tile([128, 128], f32)
            make_identity(nc, id16[:])
            make_identity(nc, id32[:])
            cones = cpool.tile([128, 256], bf16)
            nc.gpsimd.memset(cones[:], 0.0)
            nc.gpsimd.memset(cones[:, 127:128], 1.0)

            gidx_t = cpool.tile([128, 8 * (SLOTS // 16)], i16, tag="gidx")
            pidx_t = cpool.tile([128, 8 * (NP2 // 16)], i16, tag="pidx")
            umask_t = cpool.tile([BP, S], f32, tag="umask")
            imask_t = cpool.tile([BP, S], f32, tag="imask")
            in_loads = [
                nc.sync.dma_start(out=gidx_t[:], in_=gidx[:]),
                nc.sync.dma_start(out=pidx_t[:], in_=pidx[:]),
                nc.sync.dma_start(out=umask_t[:], in_=umask[:]),
                nc.sync.dma_start(out=imask_t[:], in_=imask[:]),
            ]
            sem_clears = [nc.gpsimd.sem_clear(s) for s in gsem + dsem]

            ug = epool.tile([128, 2 * BP, D], bf16, tag="ug")
            ig = epool.tile([128, 2 * BP, D], bf16, tag="ig")

            wd_of = {}
            for q in range(4):
                for side, dst in ((0, ug), (1, ig)):
                    qs = q * 2 + side
                    stg = stgpool.tile([128, SLOTS // 128, D], bf16, tag="stg")
                    gs = []
                    for c in range(4):
                        wbase = 32768 * c
                        wend = min(32768 * (c + 1), V + 1)
                        g = nc.gpsimd.dma_gather(
                            stg[:, _CWS[c] // 128:_CWS[c + 1] // 128, :],
                            table[wbase:wend, :],
                            gidx_t[:, qs * (SLOTS // 16) + _CWS[c] // 16:
                                   qs * (SLOTS // 16) + _CWS[c + 1] // 16],
                            _NCW[c], _NCW[c], D,
                        )
                        g.then_inc(gsem[qs], 16)
                        if qs == 0:
                            for cl in sem_clears:
                                add_dep(g.ins, cl.ins, sync=False, reason="clr")
                            for ld in in_loads:
                                add_dep(g.ins, ld.ins, sync=True, reason="lds")
                        gs.append(g)
                    wg = nc.gpsimd.wait_ge(gsem[qs], 16 * 4)
                    for g in gs:
                        add_dep(wg.ins, g.ins, sync=False, reason="w-after-g")
                    bounce = nc.sync.dma_start(out=scrs[qs][:], in_=stg[:])
                    add_dep(bounce.ins, wg.ins, sync=True, reason="stg data done")
                    g2 = nc.gpsimd.dma_gather(
                        dst[:, 64 * q:64 * q + 64, :],
                        scrs[qs][:].rearrange("p c d -> (p c) d"),
                        pidx_t[:, qs * (NP2 // 16):(qs + 1) * (NP2 // 16)],
                        NP2, NP2, D,
                    )
                    g2.then_inc(dsem[qs], 16)
                    wd = nc.gpsimd.wait_ge(dsem[qs], 16)
                    add_dep(wd.ins, g2.ins, sync=False, reason="w-after-g2")
                    wd_of[(side, q)] = wd

            # ---- pass 1 (v1 compute) ----
            ucc1 = coefpool.tile([128, BP], f32, tag="ucc1")
            ucc2 = coefpool.tile([128, BP], f32, tag="ucc2")
            icp = icpsum.tile([128, 512], f32, tag="ic")
            for b in range(BP):
                q = b // 32
                tp = tpsum.tile([128, 512], f32, tag="tp")
                m1 = nc.tensor.matmul(out=tp[:, 0:S1], lhsT=ug[:, 2 * b, :],
                                      rhs=id16[:], start=True, stop=True)
                m2 = nc.tensor.matmul(out=tp[:, S1:S], lhsT=ug[0:S2, 2 * b + 1, :],
                                      rhs=id16[0:S2, 0:S2], start=True, stop=True)
                m3 = nc.tensor.matmul(out=tp[:, S:S + S1], lhsT=ig[:, 2 * b, :],
                                      rhs=id16[:], start=True, stop=True)
                m4 = nc.tensor.matmul(out=tp[:, S + S1:2 * S], lhsT=ig[0:S2, 2 * b + 1, :],
                                      rhs=id16[0:S2, 0:S2], start=True, stop=True)
                for m_, sd in ((m1, 0), (m2, 0), (m3, 1), (m4, 1)):
                    add_dep(m_.ins, wd_of[(sd, q)].ins, sync=True, reason="emb rdy")
                t16 = t16pool.tile([128, 2 * S], bf16, tag="t16")
                nc.vector.tensor_copy(out=t16[:], in_=tp[:, 0:2 * S])
                ap_ = apsum.tile([128, 512], f32, tag="ap")
                nc.tensor.matmul(out=ap_[:, 0:S], lhsT=t16[:, 0:S1],
                                 rhs=t16[:, S:2 * S], start=True, stop=True)
                nc.tensor.matmul(out=ap_[0:S2, S:2 * S], lhsT=t16[:, S1:S],
                                 rhs=t16[:, S:2 * S], start=True, stop=True)
                th1 = thpool.tile([128, S], bf16, tag="th1")
                th2 = thpool.tile([S2, S], bf16, tag="th2")
                nc.scalar.activation(out=th1[:], in_=ap_[:, 0:S],
                                     func=mybir.ActivationFunctionType.Tanh,
                                     accum_out=ucc1[:, b:b + 1])
                nc.scalar.activation(out=th2[:], in_=ap_[0:S2, S:2 * S],
                                     func=mybir.ActivationFunctionType.Tanh,
                                     accum_out=ucc2[0:S2, b:b + 1])
                nc.tensor.matmul(out=icp[:, 0:S], lhsT=cones[:, 127 - b:255 - b],
                                 rhs=th1[:], start=(b == 0), stop=False,
                                 skip_group_check=True)
                nc.tensor.matmul(out=icp[:, 0:S], lhsT=cones[0:S2, 127 - b:255 - b],
                                 rhs=th2[:], start=False, stop=(b == BP - 1),
                                 skip_group_check=True)

            # ---- softmax ----
            ucp = wpsum.tile([128, 512], f32, tag="wps")
            nc.tensor.matmul(out=ucp[0:BP, 0:S1], lhsT=ucc1[:], rhs=id32[:],
                             start=True, stop=True)
            nc.tensor.matmul(out=ucp[0:BP, S1:S], lhsT=ucc2[0:S2, :],
                             rhs=id32[0:S2, 0:S2], start=True, stop=True)
            wu_sb = spool.tile([BP, S], f32, tag="wu")
            wi_sb = spool.tile([BP, S], f32, tag="wi")
            for coef_psum, mask_t, w_sb in (
                (ucp[0:BP, 0:S], umask_t, wu_sb),
                (icp[0:BP, 0:S], imask_t, wi_sb),
            ):
                y0 = spool.tile([BP, S], f32, tag="y0")
                y1 = spool.tile([BP, S], f32, tag="y1")
                mx = spool.tile([BP, 1], f32, tag="mx")
                se = spool.tile([BP, 1], f32, tag="se")
                rse = spool.tile([BP, 1], f32, tag="rse")
                nc.vector.tensor_scalar(out=y0[:], in0=coef_psum, scalar1=1.0 / S,
                                        scalar2=None, op0=mybir.AluOpType.mult)
                nc.vector.tensor_tensor(out=y1[:], in0=y0[:], in1=mask_t[:],
                                        op=mybir.AluOpType.add)
                nc.vector.reduce_max(out=mx[:], in_=y1[:], axis=mybir.AxisListType.X)
                nc.vector.tensor_scalar(out=y0[:], in0=y1[:], scalar1=mx[:],
                                        scalar2=None, op0=mybir.AluOpType.subtract)
                nc.scalar.activation(out=y1[:], in_=y0[:],
                                     func=mybir.ActivationFunctionType.Exp,
                                     accum_out=se[:])
                nc.vector.reciprocal(out=rse[:], in_=se[:])
                nc.vector.tensor_scalar(out=w_sb[:], in0=y1[:], scalar1=rse[:],
                                        scalar2=None, op0=mybir.AluOpType.mult)

            wtp = wpsum.tile([128, 512], f32, tag="wps")
            nc.tensor.matmul(out=wtp[:, 0:BP], lhsT=wu_sb[:, 0:S1],
                             rhs=id32[0:BP, 0:BP], start=True, stop=True)
            nc.tensor.matmul(out=wtp[0:S2, 128:128 + BP], lhsT=wu_sb[:, S1:S],
                             rhs=id32[0:BP, 0:BP], start=True, stop=True)
            nc.tensor.matmul(out=wtp[:, 256:256 + BP], lhsT=wi_sb[:, 0:S1],
                             rhs=id32[0:BP, 0:BP], start=True, stop=True)
            nc.tensor.matmul(out=wtp[0:S2, 384:384 + BP], lhsT=wi_sb[:, S1:S],
                             rhs=id32[0:BP, 0:BP], start=True, stop=True)
            wu1 = spool.tile([128, BP], bf16, tag="wu1")
            wu2 = spool.tile([S2, BP], bf16, tag="wu2")
            wi1 = spool.tile([128, BP], bf16, tag="wi1")
            wi2 = spool.tile([S2, BP], bf16, tag="wi2")
            nc.vector.tensor_copy(out=wu1[:], in_=wtp[:, 0:BP])
            nc.vector.tensor_copy(out=wu2[:], in_=wtp[0:S2, 128:128 + BP])
            nc.vector.tensor_copy(out=wi1[:], in_=wtp[:, 256:256 + BP])
            nc.vector.tensor_copy(out=wi2[:], in_=wtp[0:S2, 384:384 + BP])

            # ---- pass 2 ----
            repp = icpsum.tile([128, 512], f32, tag="rep")
            for b in range(BP):
                nc.tensor.matmul(out=repp[:, b:b + 1], lhsT=ug[:, 2 * b, :],
                                 rhs=wu1[:, b:b + 1], start=True, stop=False,
                                 skip_group_check=True)
                nc.tensor.matmul(out=repp[:, b:b + 1], lhsT=ug[0:S2, 2 * b + 1, :],
                                 rhs=wu2[:, b:b + 1], start=False, stop=True,
                                 skip_group_check=True)
                nc.tensor.matmul(out=repp[:, BP + b:BP + b + 1], lhsT=ig[:, 2 * b, :],
                                 rhs=wi1[:, b:b + 1], start=True, stop=False,
                                 skip_group_check=True)
                nc.tensor.matmul(out=repp[:, BP + b:BP + b + 1],
                                 lhsT=ig[0:S2, 2 * b + 1, :],
                                 rhs=wi2[:, b:b + 1], start=False, stop=True,
                                 skip_group_check=True)

            repsb = spool.tile([128, 2 * BP], f32, tag="repsb")
            nc.vector.tensor_copy(out=repsb[:], in_=repp[:, 0:2 * BP])
            outp = wpsum.tile([128, 512], f32, tag="wps")
            nc.tensor.matmul(out=outp[0:BP, 0:D], lhsT=repsb[:, 0:BP], rhs=id32[:],
                             start=True, stop=True)
            nc.tensor.matmul(out=outp[0:BP, D:2 * D], lhsT=repsb[:, BP:2 * BP],
                             rhs=id32[:], start=True, stop=True)
            outsb = spool.tile([BP, 2 * D], f32, tag="outsb")
            nc.vector.tensor_copy(out=outsb[:], in_=outp[0:BP, 0:2 * D])
            nc.sync.dma_start(out=out_t[0], in_=outsb[:, 0:D])
            nc.sync.dma_start(out=out_t[1], in_=outsb[:, D:2 * D])

    nc.finalize()
    return nc


def _pack_v3(nh_u, nh_i):
    """gidx [128, 8*448] + pidx [128, 8*512] int16 for the v3 bounce design."""
    gcols, pcols = [], []
    for q in range(4):
        for side, nh in ((0, nh_u), (1, nh_i)):
            rows = nh[32 * q:32 * q + 32].reshape(-1).astype(np.int64)
            w = rows >> 15
            counts = np.bincount(w, minlength=4)
            if (counts > np.array(_NCW)).any():
                raise ValueError("window budget overflow")
            order = np.argsort(w, kind="stable")
            slot_of = np.empty(_NQ, np.int64)
            # pad with VALID window-local index 0 (row 32768c): no negative
            # indices anywhere -> no ucode trim, descriptor count exact
            glist = np.zeros(_SLOTS, np.int64)
            pos = 0
            for c in range(4):
                cols = order[pos:pos + counts[c]]
                slots = _CWS[c] + np.arange(counts[c])
                slot_of[cols] = slots
                glist[slots] = rows[cols] - 32768 * c
                pos += counts[c]
            # scratch row of slot s (stage-1 wrote slot s at partition s%128,
            # col s//128; scratch is [128, 56, D] row-major): (s%128)*56+s//128
            scrrow_of = (slot_of % 128) * (_SLOTS // 128) + slot_of // 128
            # stage-2 position i fills (p=i%128, col=i//128); col=2r+chunk
            plist = np.zeros(8192, np.int64)
            ii = np.arange(8192)
            pp, cc = ii % 128, ii // 128
            rr, ch = cc // 2, cc % 2
            s_pos = ch * 128 + pp          # position within the batch (0..255)
            valid = s_pos < _S
            flat = rr * _S + np.minimum(s_pos, _S - 1)
            plist[valid] = scrrow_of[flat[valid]]
            plist[~valid] = 0
            gcols.append(_wrap16(glist))
            pcols.append(_wrap16(plist))
    return np.concatenate(gcols, axis=1), np.concatenate(pcols, axis=1)


def _wrap16(lst):
    n = len(lst)
    w = np.asarray(lst, np.int16).reshape(n // 16, 16).T.copy()
    return np.tile(w, (8, 1))


def _pack_v2(nh_u, nh_i):
    """Build gidx [128, 8*448] and pidx [128, 8*400] int16 for one core.
    Raises ValueError if a quarter-side window budget overflows."""
    gcols, pcols = [], []
    for q in range(4):
        for side, nh in ((0, nh_u), (1, nh_i)):
            rows = nh[32 * q:32 * q + 32].reshape(-1).astype(np.int64)
            w = rows >> 15
            counts = np.bincount(w, minlength=4)
            if (counts > np.array(_NCW)).any():
                raise ValueError("window budget overflow")
            order = np.argsort(w, kind="stable")
            slot_of = np.empty(_NQ, np.int64)
            glist = np.full(_SLOTS, -1, np.int64)
            pos = 0
            for c in range(4):
                cols = order[pos:pos + counts[c]]
                slots = _CWS[c] + np.arange(counts[c])
                slot_of[cols] = slots
                glist[slots] = rows[cols] - 32768 * c
                pos += counts[c]
            gcols.append(_wrap16(glist))
            pcols.append(_wrap16(slot_of))
    return np.concatenate(gcols, axis=1), np.concatenate(pcols, axis=1)


def _pack_indices(nh_shard):
    """[BP, S] int -> [128, 2*BP] int32 gather-index layout."""
    BP = nh_shard.shape[0]
    idx = np.full((128, 2 * BP), _PAD, np.int32)
    idx[:, 0::2] = nh_shard[:, 0:128].T
    idx[0:72, 1::2] = nh_shard[:, 128:200].T
    return idx


def _to_bf16(x):
    import ml_dtypes
    return np.asarray(x, np.float32).astype(ml_dtypes.bfloat16)


def kernel(users, user_nh, user_mask, items, item_nh, item_mask, emb_table):
    from concourse.bass_utils import run_bass_kernel_spmd

    user_nh = np.asarray(user_nh)
    item_nh = np.asarray(item_nh)
    user_mask = np.asarray(user_mask, dtype=np.float32)
    item_mask = np.asarray(item_mask, dtype=np.float32)
    tbl = np.array(emb_table, dtype=np.float32, copy=True)
    tbl[0] = 0.0  # padding_idx
    tbl16 = _to_bf16(tbl)

    import os
    # v1 (per-partition indirect gathers) is the default: validated on HW.
    # v2 (bulk two-stage dma_gather, ~5x faster projected) passes CoreSim but
    # still hits a device-side DMA issue on HW; opt in with BASS_KERNEL_V=2.
    variant = os.environ.get("BASS_KERNEL_V", "1")
    in_maps, nc = [], None
    if variant == "3":
        try:
            for c in range(_NC):
                sl = slice(c * _BP, (c + 1) * _BP)
                g, p = _pack_v3(user_nh[sl].astype(np.int64),
                                item_nh[sl].astype(np.int64))
                in_maps.append({
                    "table": tbl16,
                    "gidx": g,
                    "pidx": p,
                    "umask": np.ascontiguousarray(user_mask[sl]) * np.float32(_S),
                    "imask": np.ascontiguousarray(item_mask[sl]) * np.float32(_S),
                })
            nc = build_bass_v3()
        except ValueError:
            in_maps, nc = [], None
    if variant == "2":
        try:
            for c in range(_NC):
                sl = slice(c * _BP, (c + 1) * _BP)
                g, p = _pack_v2(user_nh[sl].astype(np.int64),
                                item_nh[sl].astype(np.int64))
                in_maps.append({
                    "table": tbl16,
                    "gidx": g,
                    "pidx": p,
                    "umask": np.ascontiguousarray(user_mask[sl]) * np.float32(_S),
                    "imask": np.ascontiguousarray(item_mask[sl]) * np.float32(_S),
                })
            nc = build_bass_v2()
        except ValueError:
            in_maps, nc = [], None  # window-budget overflow -> v1 fallback

    if nc is None:
        in_maps = []
        for c in range(_NC):
            sl = slice(c * _BP, (c + 1) * _BP)
            in_maps.append({
                "table": tbl16,
                "uidx": _pack_indices(user_nh[sl].astype(np.int64)),
                "iidx": _pack_indices(item_nh[sl].astype(np.int64)),
                "umask": np.ascontiguousarray(user_mask[sl]) * np.float32(_S),
                "imask": np.ascontiguousarray(item_mask[sl]) * np.float32(_S),
            })
        nc = build_bass()

    extra = {}
    tdir = globals().get("TRACE_TMPDIR")
    if tdir:
        extra["tmpdir"] = tdir
    res = run_bass_kernel_spmd(nc, in_maps, core_ids=list(range(_NC)), **extra)
    kernel.last_exec_time_ns = res.exec_time_ns
    out = np.concatenate([r["out"] for r in res.results], axis=1)
    return out.astype(np.float32)


kernel.last_exec_time_ns = None

